# revision 22
# baseline (speedup 1.0000x reference)
"""Bass/Trainium2 kernel for nn_Attention_66297115181568 (sparse_attention).

Strategy: head-parallel across 8 NeuronCores. Core h computes head h
end-to-end; the host sums the 8 partial (512, 512) outputs (the
tensor-parallel all-reduce done at unshard time) and adds bo.

The dominant cost is streaming rel_pos (67MB/core in f32). Two changes
vs the v1 kernel:
  1. rel_pos is cast to fp16 and transposed to (i-tile, pair, k, j)
     layout ON THE HOST: DMA bytes halve to 33.5MB/core (~94us at the
     ~356GB/s per-core HBM ceiling).
  2. The rel contraction relterm[i,j] = sum_d q'[i,d]*rel[i,j,d] moves
     from DVE/ACT/GpSimd onto the idle tensor engine: for each pair of
     rows (i0=base+p, i1=base+64+p) a 128x128 stationary holds q'[i0]
     in column p (k rows 64:128) and q'[i1] in column p+64 (k rows
     0:64), zeros elsewhere; the moving operand is the (128, 512) rel
     tile with both rows' d-vectors stacked on partitions. 64 such
     matmuls accumulate relterm directly into the QK^T PSUM tile (zero
     columns contribute nothing), so no separate relterm buffer, adds,
     or reductions exist at all.

Per-core engine plan:
  PE    : q/k/v projections (fp16), q'/k' transposes, QK^T+mask (one
          k=65 matmul), 64 rel pair-matmuls per row tile, w transposes,
          AV, Wo partial.
  ACT   : PSUM->SBUF copies (with f32->fp16 casts), exp with fused
          row-sum.
  DVE   : RoPE, block-diag stationary strided writes, softmax
          max/normalize.
  DMA   : rel shard streamed as 32 x 1MB tiles, fully overlapped.
"""

import sys

sys.path.insert(0, "/opt/trn_rl_repo")

from contextlib import ExitStack

import numpy as np

import concourse.bass as bass
import concourse.tile as tile
from concourse import mybir
from concourse.masks import make_identity

# problem dims (hardcoded per spec)
B, N, DIM, H, D = 1, 512, 512, 8, 64
INNER = H * D
N_CORES = 8
P = 128                 # SBUF partitions
NT = N // P             # 4 row tiles
KT = DIM // P           # 4 contraction tiles
NPAIR = P // 2          # 64 row pairs per row tile
G = 8                   # pairs per rel DMA (128p x 8KB = 1MB)
NG = NPAIR // G         # 4 DMA groups per row tile
SCALE = D ** -0.5
MASK_BIG = 60000.0      # fp16-safe additive mask magnitude

f32 = mybir.dt.float32
f16 = mybir.dt.float16
u8 = mybir.dt.uint8
AX = mybir.AxisListType
ALU = mybir.AluOpType
AF = mybir.ActivationFunctionType


def legalize_multi_waits(nc):
    """This walrus build supports only one sync-wait per instruction; hoist
    extra waits onto same-engine NoOps placed immediately before."""
    nid = 0
    for fn in nc.m.functions:
        for bb in fn.blocks:
            new = []
            changed = False
            for inst in bb.instructions:
                si = inst.sync_info
                waits = si.on_wait if si is not None else []
                if len(waits) > 1:
                    for w in waits[:-1]:
                        nop = mybir.InstNoOp(name=f"I-waitfix-{nid}")
                        nid += 1
                        nop.engine = inst.engine
                        nop.sync_info = mybir.SyncInfo(on_wait=[w], on_update=[])
                        new.append(nop)
                    si.on_wait = [waits[-1]]
                    inst.sync_info = si
                    changed = True
                new.append(inst)
            if changed:
                bb.instructions = new


def build_nc():
    nc = bass.Bass()

    xT_ext = nc.declare_dram_parameter("xT", [DIM, N], f16, isOutput=False)
    wqkv_ext = nc.declare_dram_parameter("wqkv", [DIM, 3 * D], f16, isOutput=False)
    bqkv_ext = nc.declare_dram_parameter("bqkv", [1, 3 * D], f16, isOutput=False)
    wo_ext = nc.declare_dram_parameter("wo", [D, DIM], f16, isOutput=False)
    rope_ext = nc.declare_dram_parameter("rope", [N, D], f32, isOutput=False)
    mask_ext = nc.declare_dram_parameter("mask", [1, N], u8, isOutput=False)
    rel_ext = nc.declare_dram_parameter("rel", [NT, NG, P, G, N], f16,
                                        isOutput=False)
    out_ext = nc.declare_dram_parameter("out", [N, DIM], f32, isOutput=True)

    with tile.TileContext(nc) as tc, ExitStack() as ctx:
        dma = nc.sync      # HWDGE; inputs first, then the rel_pos stream
        dma2 = nc.scalar   # HWDGE; outputs (kept off the rel stream ring)
        consts = ctx.enter_context(tc.tile_pool(name="consts", bufs=1))
        # PSUM: 8 banks -- 2 transpose + 2 dots + 1 av + 2 out/proj
        psum_t = ctx.enter_context(
            tc.tile_pool(name="psum_t", bufs=2, space=bass.MemorySpace.PSUM))
        psum_mm = ctx.enter_context(
            tc.tile_pool(name="psum_mm", bufs=2, space=bass.MemorySpace.PSUM))
        psum_av = ctx.enter_context(
            tc.tile_pool(name="psum_av", bufs=1, space=bass.MemorySpace.PSUM))
        psum_o = ctx.enter_context(
            tc.tile_pool(name="psum_o", bufs=1, space=bass.MemorySpace.PSUM))
        psum_p = ctx.enter_context(
            tc.tile_pool(name="psum_p", bufs=2, space=bass.MemorySpace.PSUM))
        pro = ctx.enter_context(tc.tile_pool(name="pro", bufs=1))
        relp = ctx.enter_context(tc.tile_pool(name="relp", bufs=6))
        sm = ctx.enter_context(tc.tile_pool(name="sm", bufs=2))
        outp = ctx.enter_context(tc.tile_pool(name="outp", bufs=2))

        # ---- constants ----
        # block-diagonal stationaries: qbd[k, it, pair, col]; zeros persist,
        # only the two diagonals are rewritten per row tile. Tile 0's zeros
        # are the first DVE op (critical path); tiles 1-3 are zeroed inside
        # the prologue loop so they overlap the rel stream.
        qbd = consts.tile([P, NT, NPAIR, P], f16)
        qbd_z = qbd.rearrange("p t a b -> p t (a b)")
        nc.vector.memset(qbd_z[:, 0, :], 0.0)
        ident = consts.tile([P, P], f32)
        make_identity(nc, ident[:])
        ones_col = consts.tile([1, P], f16)
        nc.vector.memset(ones_col, 1.0)
        ident16 = consts.tile([D, D], f16)
        nc.vector.tensor_copy(ident16[:], ident[0:D, 0:D])

        # ---- load small inputs (scalar ring, proj inputs first) ----
        x_sb = pro.tile([P, KT, N], f16)         # xT: partition = k % 128
        dma2.dma_start(out=x_sb[:], in_=xT_ext.rearrange("(u p) n -> p u n", p=P))
        w_sb = pro.tile([P, KT, 3 * D], f16)
        dma2.dma_start(out=w_sb[:], in_=wqkv_ext.rearrange("(u p) n -> p u n", p=P))
        b_sb = pro.tile([1, 3 * D], f16)
        dma2.dma_start(out=b_sb[:], in_=bqkv_ext[:])
        rope_sb = pro.tile([P, NT, D], f32)
        dma2.dma_start(out=rope_sb[:], in_=rope_ext.rearrange("(t p) d -> p t d", p=P))
        wo_sb = consts.tile([D, DIM], f16)
        dma2.dma_start(out=wo_sb[:], in_=wo_ext[:])

        # q'^T / k'^T with an extra row 64: ones (q side) x additive mask row
        # (k side) so one k=65 matmul computes QK^T + mask bias.
        qpT16 = consts.tile([D + 1, N], f16)
        kpT16 = consts.tile([D + 1, N], f16)
        nc.vector.memset(qpT16[D:D + 1, :], 1.0)
        masku8 = pro.tile([D + 1, N], u8)
        dma2.dma_start(out=masku8[D:D + 1, :], in_=mask_ext[:])
        maskf = pro.tile([D + 1, N], f32)
        nc.vector.tensor_copy(maskf[D:D + 1, :], masku8[D:D + 1, :])
        nc.vector.tensor_scalar(kpT16[D:D + 1, :], maskf[D:D + 1, :],
                                MASK_BIG, -MASK_BIG, ALU.mult, ALU.add)

        # ---- cos/sin (ACT): cos(x) = sin(x + pi/2); fold QK scale into k's ----
        sin_sb = pro.tile([P, NT, D], f32)
        cos_sb = pro.tile([P, NT, D], f32)
        halfpi = consts.tile([P, 1], f32)
        nc.vector.memset(halfpi, float(np.pi / 2))
        nc.scalar.activation(sin_sb[:], rope_sb[:], AF.Sin)
        nc.scalar.activation(cos_sb[:], rope_sb[:], AF.Sin, bias=halfpi[:])
        sink_sb = pro.tile([P, NT, D], f32)
        cosk_sb = pro.tile([P, NT, D], f32)
        nc.vector.tensor_scalar_mul(sink_sb[:], sin_sb[:], SCALE)
        nc.vector.tensor_scalar_mul(cosk_sb[:], cos_sb[:], SCALE)

        # ---- per-row-tile prologue: q/k/v proj -> RoPE -> transposes ----
        q_sb = pro.tile([P, NT, D], f32)
        k_sb = pro.tile([P, NT, D], f32)
        v_sb = consts.tile([P, NT, D], f16)
        qp_sb = pro.tile([P, NT, D], f32)
        kp_sb = pro.tile([P, NT, D], f32)
        qbd_flat = qbd.rearrange("p t a b -> p (t a b)")
        for t in range(NT):
            if t >= 1:
                # zeros for tile t's stationaries; WAW-ordered before the
                # diagonal writes below because it is emitted first
                nc.vector.memset(qbd_z[:, t, :], 0.0)
            ps = psum_p.tile([P, 3 * D], f32, tag="proj")
            for u in range(KT):
                nc.tensor.matmul(ps[:], x_sb[:, u, t * P:(t + 1) * P],
                                 w_sb[:, u, :], start=(u == 0), stop=False)
            nc.tensor.matmul(ps[:], ones_col[:, 0:P], b_sb[:],
                             start=False, stop=True)
            nc.scalar.copy(q_sb[:, t, :], ps[:, 0:D])
            nc.scalar.copy(k_sb[:, t, :], ps[:, D:2 * D])
            nc.scalar.copy(v_sb[:, t, :], ps[:, 2 * D:3 * D])
            for (src, dst, c, s) in ((q_sb, qp_sb, cos_sb, sin_sb),
                                     (k_sb, kp_sb, cosk_sb, sink_sb)):
                sr = src[:, t, :].rearrange("p (m two) -> p m two", two=2)
                rot = pro.tile([P, D // 2, 2], f32, tag="rot")
                nc.vector.tensor_scalar_mul(rot[:, :, 0], sr[:, :, 1], -1.0)
                nc.vector.tensor_copy(rot[:, :, 1], sr[:, :, 0])
                tmp = pro.tile([P, D], f32, tag="ropetmp")
                nc.vector.tensor_mul(tmp[:], rot.rearrange("p m two -> p (m two)"),
                                     s[:, t, :])
                nc.vector.tensor_mul(dst[:, t, :], src[:, t, :], c[:, t, :])
                nc.vector.tensor_add(dst[:, t, :], dst[:, t, :], tmp[:])
            # full q' transpose (64, 128) at psum base 0
            ps1 = psum_t.tile([P, P], f32, tag="tp")
            nc.tensor.transpose(ps1[0:D, :], qp_sb[:, t, :], ident[:])
            nc.scalar.copy(qpT16[0:D, t * P:(t + 1) * P], ps1[0:D, :])
            # half transpose of q' rows 0:64 (even group) at psum base 64.
            # Transpose-mode matmuls must write psum partition 0, so use a
            # regular matmul (qp16 as stationary x identity) col-tiled to 64.
            qp16 = pro.tile([D, D], f16, tag="qp16")
            nc.vector.tensor_copy(qp16[:], qp_sb[0:D, t, :])
            ps2 = psum_t.tile([P, P], f32, tag="tp")
            nc.tensor.matmul(ps2[D:P, 0:D], qp16[:], ident16[:],
                             start=True, stop=True)
            # stationary diagonals for this row tile (f32->f16 strided writes):
            # col p+64 (k 0:64)  <- q'[base+64+p];  col p (k 64:128) <- q'[base+p]
            base = t * NPAIR * P
            nc.vector.tensor_copy(
                qbd_flat[0:D, base + D:base + NPAIR * P:P + 1],
                ps1[0:D, D:P])
            nc.vector.tensor_copy(
                qbd_flat[D:P, base:base + NPAIR * P:P + 1],
                ps2[D:P, 0:D])
            # k' transpose
            ps3 = psum_t.tile([P, P], f32, tag="tp")
            nc.tensor.transpose(ps3[0:D, :], kp_sb[:, t, :], ident[:])
            nc.scalar.copy(kpT16[0:D, t * P:(t + 1) * P], ps3[0:D, :])

        # ---- main loop over row tiles ----
        def softmax_av_out(it, dots_ps):
            rowmax = sm.tile([P, 1], f32, tag="rowmax")
            nc.vector.tensor_reduce(rowmax[:], dots_ps[:], AX.X, ALU.max)
            negmax = sm.tile([P, 1], f32, tag="negmax")
            nc.vector.tensor_scalar_mul(negmax[:], rowmax[:], -1.0)
            w_sm = sm.tile([P, N], f32, tag="w_sm")
            rowsum = sm.tile([P, 1], f32, tag="rowsum")
            nc.scalar.activation(w_sm[:], dots_ps[:], AF.Exp, bias=negmax[:],
                                 accum_out=rowsum[:])
            rcp = sm.tile([P, 1], f32, tag="rcp")
            nc.vector.reciprocal(rcp[:], rowsum[:])
            wT16 = outp.tile([P, NT, P], f16, tag="wT16")
            for jt in range(NT):
                wT_ps = psum_t.tile([P, P], f32, tag="tp")
                nc.tensor.transpose(wT_ps[:], w_sm[:, jt * P:(jt + 1) * P], ident[:])
                nc.scalar.copy(wT16[:, jt, :], wT_ps[:])
            attn_ps = psum_av.tile([D, P], f32, tag="attn")
            for jt in range(NT):
                nc.tensor.matmul(attn_ps[:], v_sb[:, jt, :], wT16[:, jt, :],
                                 start=(jt == 0), stop=(jt == NT - 1))
            attn16 = outp.tile([D, P], f16, tag="attn16")
            nc.scalar.copy(attn16[:], attn_ps[:])
            out_ps = psum_o.tile([P, DIM], f32, tag="out_ps")
            nc.tensor.matmul(out_ps[:], attn16[:], wo_sb[:], start=True, stop=True)
            # softmax normalization folded in here: rows scale by 1/rowsum
            o_sb = outp.tile([P, DIM], f32, tag="o_sb")
            nc.vector.tensor_scalar_mul(o_sb[:], out_ps[:], rcp[:])
            dma2.dma_start(out=out_ext[it * P:(it + 1) * P, :], in_=o_sb[:])

        gate_srcs = [x_sb[0:1, 0, 0:1], rope_sb[0:1, 0, 0:1]] + \
            [masku8[D:D + 1, 0:1]] * 4
        for it in range(NT):
            dots_ps = psum_mm.tile([P, N], f32, tag="mm")
            # rel pair-matmuls first (start=True on pair 0 clears the bank),
            # QK^T + mask joins last so it never gates the stream.
            for gr in range(NG):
                rl = relp.tile([P, G, N], f16)
                if it == 0 and gr < len(gate_srcs):
                    # delay the first in-flight rel tiles behind the (small)
                    # input DMAs: the tile scheduler does not keep ring order
                    nc.gpsimd.tensor_copy(rl[0:1, 0:1, 0:1], gate_srcs[gr])
                dma.dma_start(out=rl[:], in_=rel_ext[it, gr])
                for g8 in range(G):
                    pr = gr * G + g8
                    nc.tensor.matmul(dots_ps[:], qbd[:, it, pr, :], rl[:, g8, :],
                                     start=(pr == 0), stop=False)
            nc.tensor.matmul(dots_ps[:], qpT16[:, it * P:(it + 1) * P], kpT16[:],
                             start=False, stop=True)
            softmax_av_out(it, dots_ps)

    legalize_multi_waits(nc)
    return nc


_NC_CACHE = None
TRACE = False        # set by test harness to capture an NTFF profile
LAST_RESULT = None   # BassKernelResults of the most recent kernel() call


def _get_nc():
    global _NC_CACHE
    if _NC_CACHE is None:
        _NC_CACHE = build_nc()
    return _NC_CACHE


def _repack_rel(rel_h):
    """(N, N, D) f32 -> (NT, NG, 2*D, G, N) fp16 pair layout: each (it, gr)
    DMA group is one fully contiguous DRAM block, partition k major inside;
    k rows 0:64 hold d of i1=base+64+p (odd group), 64:128 of i0=base+p."""
    r = rel_h.transpose(0, 2, 1)                       # (i, d, j)
    rh = r.reshape(NT, 2, NPAIR, D, N)                 # (it, g, p, d, j)
    a = rh[:, ::-1].transpose(0, 1, 3, 2, 4).reshape(NT, P, NPAIR, N)
    a = a.reshape(NT, P, NG, G, N).transpose(0, 2, 1, 3, 4)
    return np.ascontiguousarray(a, dtype=np.float16)


def kernel(**inputs):
    x = np.asarray(inputs["x"], dtype=np.float32)
    mask = np.asarray(inputs["mask"])
    rope = np.asarray(inputs["rope"], dtype=np.float32)
    rel_pos = np.asarray(inputs["rel_pos"], dtype=np.float32)
    Wq = np.asarray(inputs["Wq"], dtype=np.float32)
    bq = np.asarray(inputs["bq"], dtype=np.float32)
    Wk = np.asarray(inputs["Wk"], dtype=np.float32)
    bk = np.asarray(inputs["bk"], dtype=np.float32)
    Wv = np.asarray(inputs["Wv"], dtype=np.float32)
    bv = np.asarray(inputs["bv"], dtype=np.float32)
    Wo = np.asarray(inputs["Wo"], dtype=np.float32)
    bo = np.asarray(inputs["bo"], dtype=np.float32)

    nc = _get_nc()

    xT = np.ascontiguousarray(x.reshape(N, DIM).T).astype(np.float16)
    mask_u8 = np.ascontiguousarray(mask.reshape(1, N).astype(np.uint8, copy=False))
    rope2 = np.ascontiguousarray(rope)

    in_maps = []
    for h in range(N_CORES):
        sl = slice(h * D, (h + 1) * D)
        wqkv = np.concatenate([Wq[:, sl], Wk[:, sl], Wv[:, sl]],
                              axis=1).astype(np.float16)
        bqkv = np.concatenate([bq[sl], bk[sl], bv[sl]])[None, :].astype(np.float16)
        in_maps.append({
            "xT": xT,
            "wqkv": np.ascontiguousarray(wqkv),
            "bqkv": np.ascontiguousarray(bqkv),
            "wo": np.ascontiguousarray(Wo[sl, :]).astype(np.float16),
            "rope": rope2,
            "mask": mask_u8,
            "rel": _repack_rel(rel_pos[0, h]),
        })

    from concourse.bass_utils import run_bass_kernel_spmd
    res = run_bass_kernel_spmd(nc, in_maps, list(range(N_CORES)), trace=TRACE)
    globals()["LAST_RESULT"] = res
    out = np.zeros((N, DIM), dtype=np.float32)
    for h in range(N_CORES):
        out += res.results[h]["out"]
    out += bo[None, :]
    return out.reshape(B, N, DIM)
